# revision 1
# baseline (speedup 1.0000x reference)
"""Trainium2 Bass kernel for nn_AdaptiveEmbeddingI2T.

Computes, for image-batch shard i on each of 8 NeuronCores:
  sims[i, b] = <img_vec_i, txt_vec_ib> with
  txt_vec_ib = l2norm_d( mean_t( softmax_t(10*(gam_id*xn_bdt+bet_id)) * (gam*xn+bet) ) )

Device-side algebra (softmax shift-invariance folds BN into an affine):
  exponent = es*cap + eb with es = 10*gam*rs, eb = -es*mu
  txt_vec = wscale*(S2/S1) + wbias, S1 = sum_t e, S2 = sum_t e*cap
  sims = (sum_d v*w) * rsqrt(sum_d w^2) * rsqrt(sum_d v^2)

Layout: cap arrives pre-transposed/cast from host as bf16 [d%128, (dt,t,b)]
(t-major columns). Per (i,dt) the main loop computes e (ACT Exp) and
q=e*cap (DVE) side by side in [e|q] 128-col slabs, then a contiguous
slab-halving add tree (first level split DVE/GPSIMD) yields [S1|S2].
BN stats: sum via DVE halving-tree, sum-of-squares on ACT Square+accum
(tensor_tensor_reduce is avoided: it hard-crashes real TRN2 HW).

Sharding: image batch axis across 8 cores (8 images/core); cap + params
replicated; host concatenates the (8, 64) row blocks.
"""

import sys

if "/opt/trn_rl_repo" not in sys.path:
    sys.path.insert(0, "/opt/trn_rl_repo")

import numpy as np

import concourse.bacc as bacc
import concourse.mybir as mybir
from concourse.bass_utils import run_bass_kernel_spmd
from concourse.tile import TileContext

B_IMG, B_CAP, T_CAP, D = 64, 64, 64, 1024
H = 128
T_IMG = 36
EPS = 1e-5
N_CORES = 8
BI = B_IMG // N_CORES          # images per core
R = B_CAP * T_IMG              # 2304 caption cols per dt (t-major: col = t*64+b)
NDT = D // 128                 # 8 channel tiles

F32 = mybir.dt.float32
BF16 = mybir.dt.bfloat16

_COMPILED = None


def _build():
    nc = bacc.Bacc("TRN2", target_bir_lowering=False, debug=False,
                   num_devices=N_CORES)
    tensors = _declare_io(nc)
    with TileContext(nc) as tc:
        _emit(nc, tc, *tensors)
    nc.compile()
    return nc


def _emit(nc, tc, capT_d, img_d, wg1_d, wg2_d, wb1_d, wb2_d,
          bg1_d, bb1_d, bg2p1_d, bb2_d, o36_d, out_d, reps_main=1):
    AF = mybir.ActivationFunctionType
    ALU = mybir.AluOpType
    AX = mybir.AxisListType
    import os
    import contextlib
    _stage = int(os.environ.get("KSTAGE", "7"))
    _gs = int(os.environ.get("KGS", "1536"))      # L1 cols on gpsimd
    _wb = int(os.environ.get("KWBUFS", "4"))
    ctx = contextlib.ExitStack()
    with ctx:
        const = ctx.enter_context(tc.tile_pool(name="const", bufs=1))
        stream = ctx.enter_context(tc.tile_pool(name="stream", bufs=2))
        imgs = ctx.enter_context(tc.tile_pool(name="imgs", bufs=2))
        work = ctx.enter_context(tc.tile_pool(name="work", bufs=_wb))
        small = ctx.enter_context(tc.tile_pool(name="small", bufs=1))
        actx = ctx.enter_context(contextlib.ExitStack())
        ppool = actx.enter_context(tc.tile_pool(name="psum", bufs=2, space="PSUM"))
        pacc = actx.enter_context(tc.tile_pool(name="pacc", bufs=1, space="PSUM"))

        # ---- small params ----
        bg1_s = const.tile([H, 1], F32, tag="bg1")
        nc.sync.dma_start(out=bg1_s[:], in_=bg1_d[:])
        bb1_s = const.tile([H, 1], F32, tag="bb1")
        nc.sync.dma_start(out=bb1_s[:], in_=bb1_d[:])
        bg2p1_s = const.tile([128, NDT], F32, tag="bg2p1")
        nc.sync.dma_start(out=bg2p1_s[:], in_=bg2p1_d[:])
        bb2_s = const.tile([128, NDT], F32, tag="bb2t")
        nc.sync.dma_start(out=bb2_s[:], in_=bb2_d[:])
        o36_s = const.tile([T_IMG, 1], F32, tag="o36")
        nc.sync.dma_start(out=o36_s[:], in_=o36_d[:])

        # ---- MLP weights: stream f32 -> resident bf16 ----
        wg1_b = const.tile([128, NDT * H], BF16, tag="wg1b")
        wb1_b = const.tile([128, NDT * H], BF16, tag="wb1b")
        wg2_b = const.tile([128, D], BF16, tag="wg2b")
        wb2_b = const.tile([128, D], BF16, tag="wb2b")
        for w_d, w_b, view in ((wg1_d, wg1_b, True), (wb1_d, wb1_b, True),
                               (wg2_d, wg2_b, False), (wb2_d, wb2_b, False)):
            ws = stream.tile([128, D], F32, tag="stream")
            if view:
                nc.sync.dma_start(
                    out=ws[:].rearrange("p (c h) -> p c h", c=NDT),
                    in_=w_d[:].rearrange("(c p) h -> p c h", p=128))
            else:
                nc.sync.dma_start(out=ws[:], in_=w_d[:])
            nc.gpsimd.tensor_copy(w_b[:], ws[:])

        ones_b = const.tile([128, 1], BF16, tag="onesb")
        nc.gpsimd.memset(ones_b[:], 1.0)

        # ---- image means from host-transposed img: one DVE reduce ----
        imgTb = imgs.tile([128, NDT * BI * T_IMG], F32, tag="imgTb", bufs=1)
        _nic = int(os.environ.get("KIMGC", "1"))  # img DMA chunks
        _icw = (NDT * BI * T_IMG) // _nic
        for k in range(_nic):
            nc.sync.dma_start(out=imgTb[:, k * _icw:(k + 1) * _icw],
                              in_=img_d[:, k * _icw:(k + 1) * _icw])

        imgrT = const.tile([128, NDT * BI], F32, tag="imgrT")
        imgrTb = const.tile([128, NDT * BI], BF16, tag="imgrTb")
        imgrT3 = imgrT[:].rearrange("p (c i) -> p c i", c=NDT)
        imgrTb3 = imgrTb[:].rearrange("p (c i) -> p c i", c=NDT)
        nc.vector.tensor_reduce(
            imgrT[:],
            imgTb[:].rearrange("p (g t) -> p g t", t=T_IMG),
            axis=mybir.AxisListType.X, op=ALU.add)
        nc.vector.tensor_scalar_mul(imgrT[:], imgrT[:], 1.0 / T_IMG)
        nc.scalar.copy(imgrTb[:], imgrT[:])

        # ---- cap: DMA per dt chunk, stats pipelined behind chunks ----
        capTb = const.tile([128, NDT * R], BF16, tag="capTb")  # 4.6 MB
        capT3 = capTb[:].rearrange("p (c r) -> p c r", c=NDT)
        capTd3 = capT_d[:].rearrange("p (c r) -> p c r", c=NDT)
        for dt in range(NDT):
            nc.sync.dma_start(out=capT3[:, dt, :], in_=capTd3[:, dt, :])

        AXX = mybir.AxisListType.X
        musum = small.tile([128, NDT], F32, tag="musum")
        sqsum = small.tile([128, NDT], F32, tag="sqsum")
        mu = small.tile([128, NDT], F32, tag="mu")
        rs = small.tile([128, NDT], F32, tag="rs")
        tv = small.tile([128, NDT], F32, tag="tv")

        def stats_for(dt, mu_on_act=False):
            # Sum(c): DVE halving-tree (or ACT Copy+accum when deferred
            # into the main loop); Sum(c^2) via ACT Square+accum
            if mu_on_act:
                scr0 = stream.tile([128, R], BF16, tag="scr2", bufs=1)
                nc.scalar.activation(scr0[:], capT3[:, dt, :], AF.Copy,
                                     accum_out=musum[:, dt:dt + 1])
                scr2b = stream.tile([128, R], BF16, tag="scr2", bufs=1)
                nc.scalar.activation(scr2b[:], capT3[:, dt, :], AF.Square,
                                     accum_out=sqsum[:, dt:dt + 1])
                return
            st1 = stream.tile([128, 1152], BF16, tag="st1")
            nc.vector.tensor_tensor(st1[:], capT3[:, dt, 0:1152],
                                    capT3[:, dt, 1152:2304], op=ALU.add)
            st2 = stream.tile([128, 576], BF16, tag="st2")
            nc.vector.tensor_tensor(st2[:], st1[:, 0:576], st1[:, 576:1152],
                                    op=ALU.add)
            st3 = stream.tile([128, 288], BF16, tag="st3")
            nc.vector.tensor_tensor(st3[:], st2[:, 0:288], st2[:, 288:576],
                                    op=ALU.add)
            st4 = stream.tile([128, 144], BF16, tag="st4")
            nc.vector.tensor_tensor(st4[:], st3[:, 0:144], st3[:, 144:288],
                                    op=ALU.add)
            st5 = stream.tile([128, 72], BF16, tag="st5")
            nc.vector.tensor_tensor(st5[:], st4[:, 0:72], st4[:, 72:144],
                                    op=ALU.add)
            st6 = stream.tile([128, 36], BF16, tag="st6")
            nc.vector.tensor_tensor(st6[:], st5[:, 0:36], st5[:, 36:72],
                                    op=ALU.add)
            nc.vector.tensor_reduce(
                musum[:, dt:dt + 1].rearrange("p (u o) -> p u o", u=1),
                st6[:].rearrange("p (u t) -> p u t", u=1), axis=AXX,
                op=ALU.add)
            scr2 = stream.tile([128, R], BF16, tag="scr2", bufs=1)
            nc.scalar.activation(scr2[:], capT3[:, dt, :], AF.Square,
                                 accum_out=sqsum[:, dt:dt + 1])

        def stats_fin(d0, d1):
            # var = E[x^2] - mu^2 ; rs = 1/sqrt(var+eps), dt range [d0, d1)
            nc.vector.tensor_scalar_mul(mu[:, d0:d1], musum[:, d0:d1],
                                        1.0 / R)
            nc.vector.tensor_tensor(tv[:, d0:d1], mu[:, d0:d1], mu[:, d0:d1],
                                    op=ALU.mult)
            nc.vector.tensor_scalar(sqsum[:, d0:d1], sqsum[:, d0:d1],
                                    1.0 / R, None, op0=ALU.mult)
            nc.vector.tensor_tensor(tv[:, d0:d1], sqsum[:, d0:d1],
                                    tv[:, d0:d1], op=ALU.subtract)
            nc.vector.tensor_scalar_add(tv[:, d0:d1], tv[:, d0:d1], EPS)
            nc.scalar.sqrt(tv[:, d0:d1], tv[:, d0:d1])
            nc.vector.reciprocal(rs[:, d0:d1], tv[:, d0:d1])

        # stats for the first half of dts, then MLPs, then first-half prep
        _shead = int(os.environ.get("KSHEAD", "5"))
        for dt in range(_shead):
            stats_for(dt)

        # ---- CBN MLPs -> per-(d,i) scales/biases ----
        wg1_b3 = wg1_b[:].rearrange("p (c h) -> p c h", c=NDT)
        wb1_b3 = wb1_b[:].rearrange("p (c h) -> p c h", c=NDT)

        def mlp_head(w1_b3, b1_s, w2_b, b2_s, name):
            h_ps = ppool.tile([H, BI], F32, tag="tr")
            for dt in range(NDT):
                nc.tensor.matmul(h_ps[:], lhsT=w1_b3[:, dt, :],
                                 rhs=imgrTb3[:, dt, :],
                                 start=(dt == 0), stop=(dt == NDT - 1))
            hT = small.tile([H, BI], BF16, tag=f"hT_{name}")
            nc.scalar.activation(hT[:], h_ps[:], AF.Relu, bias=b1_s[:], scale=1.0)
            outT = const.tile([128, NDT * BI], F32, tag=f"outT_{name}")
            outT3 = outT[:].rearrange("p (c i) -> p c i", c=NDT)
            for dt in range(NDT):
                o_ps = ppool.tile([128, BI], F32, tag="tr")
                nc.tensor.matmul(o_ps[:], lhsT=w2_b[:, dt * 128:(dt + 1) * 128],
                                 rhs=hT[:], start=True, stop=True)
                nc.scalar.activation(outT3[:, dt, :], o_ps[:], AF.Identity,
                                     bias=b2_s[:, dt:dt + 1], scale=1.0)
            return outT3

        gamT3 = mlp_head(wg1_b3, bg1_s, wg2_b, bg2p1_s, "g")
        betT3 = mlp_head(wb1_b3, bb1_s, wb2_b, bb2_s, "b")

        # escale = 10*gam*rs ; ebias = -escale*mu
        # wscale = gam*rs/36 ; wbias = (bet - gam*rs*mu)/36
        escale = const.tile([128, NDT * BI], F32, tag="escale")
        ebias = const.tile([128, NDT * BI], F32, tag="ebias")
        wscale = const.tile([128, NDT * BI], F32, tag="wscale")
        wbias = const.tile([128, NDT * BI], F32, tag="wbias")
        es3 = escale[:].rearrange("p (c i) -> p c i", c=NDT)
        eb3 = ebias[:].rearrange("p (c i) -> p c i", c=NDT)
        ws3 = wscale[:].rearrange("p (c i) -> p c i", c=NDT)
        wb3 = wbias[:].rearrange("p (c i) -> p c i", c=NDT)
        # batched across (dt-range, i): per-dt scalars broadcast along i
        grs = stream.tile([128, NDT * BI], F32, tag="grs", bufs=1)
        tmp = stream.tile([128, NDT * BI], F32, tag="tmpb", bufs=1)
        grs3 = grs[:].rearrange("p (c i) -> p c i", c=NDT)
        tmp3 = tmp[:].rearrange("p (c i) -> p c i", c=NDT)
        rs_b = rs[:].rearrange("p (c u) -> p c u", u=1).broadcast_to(
            [128, NDT, BI])
        mu_b = mu[:].rearrange("p (c u) -> p c u", u=1).broadcast_to(
            [128, NDT, BI])

        esS = const.tile([128, NDT * BI], F32, tag="esS")
        ebS = const.tile([128, NDT * BI], F32, tag="ebS")
        esS3 = esS[:].rearrange("p (c i) -> p c i", c=NDT)
        ebS3 = ebS[:].rearrange("p (c i) -> p c i", c=NDT)

        def prep_for(d0, d1):
            nc.vector.tensor_tensor(grs3[:, d0:d1, :], gamT3[:, d0:d1, :],
                                    rs_b[:, d0:d1, :], op=ALU.mult)
            nc.vector.tensor_scalar_mul(es3[:, d0:d1, :], grs3[:, d0:d1, :],
                                        10.0)
            nc.vector.tensor_tensor(eb3[:, d0:d1, :], es3[:, d0:d1, :],
                                    mu_b[:, d0:d1, :], op=ALU.mult)
            nc.vector.tensor_scalar_mul(eb3[:, d0:d1, :], eb3[:, d0:d1, :],
                                        -1.0)
            nc.vector.tensor_scalar_mul(ws3[:, d0:d1, :], grs3[:, d0:d1, :],
                                        1.0 / 36.0)
            nc.vector.tensor_tensor(tmp3[:, d0:d1, :], grs3[:, d0:d1, :],
                                    mu_b[:, d0:d1, :], op=ALU.mult)
            nc.vector.tensor_tensor(tmp3[:, d0:d1, :], betT3[:, d0:d1, :],
                                    tmp3[:, d0:d1, :], op=ALU.subtract)
            nc.vector.tensor_scalar_mul(wb3[:, d0:d1, :], tmp3[:, d0:d1, :],
                                        1.0 / 36.0)
            # Schraudolph coefficients: bits = 184.664*z + 16249
            nc.vector.tensor_scalar_mul(esS3[:, d0:d1, :], es3[:, d0:d1, :],
                                        184.664)
            nc.vector.tensor_scalar(ebS3[:, d0:d1, :], eb3[:, d0:d1, :],
                                    184.664, 16249.0, op0=ALU.mult,
                                    op1=ALU.add)

        stats_fin(0, _shead)
        prep_for(0, _shead)

        # second half of stats/prep is interleaved into the first main-loop
        # pairs so the first exps aren't queued behind it on ACT
        _mu2act = os.environ.get("KMU2", "0") == "1"

        def _tail_stats(dt):
            def run():
                stats_for(dt, mu_on_act=_mu2act)
                if dt == NDT - 1:
                    stats_fin(_shead, NDT)
                    prep_for(_shead, NDT)
            return run

        deferred_stats = [_tail_stats(dt) for dt in range(_shead, NDT)]

        # rnorm: 1/||v_i|| via accumulating [1,1] matmuls (epilogue-only,
        # deferred so MLP matmuls run first on the cold PE)
        nrm2_ps = pacc.tile([1, BI], F32, tag="nrm2_ps")
        for i in range(BI):
            for dt in range(NDT):
                nc.tensor.matmul(
                    nrm2_ps[:, i:i + 1],
                    lhsT=imgrT3[:, dt, i:i + 1], rhs=imgrT3[:, dt, i:i + 1],
                    start=(dt == 0), stop=(dt == NDT - 1))
        nrm_row = small.tile([1, BI], F32, tag="nrm_row")
        nc.scalar.sqrt(nrm_row[:], nrm2_ps[:])
        rsr_row = small.tile([1, BI], F32, tag="rsr_row")
        nc.vector.reciprocal(rsr_row[:], nrm_row[:])
        # ---- main loop over (i, dt) ----
        actx.close()  # release phase-A psum banks
        pmain = ctx.enter_context(tc.tile_pool(name="pmain", bufs=1,
                                               space="PSUM"))
        dot_ps = pmain.tile([1, BI * B_CAP], F32, tag="dot_ps")
        nrm_ps = pmain.tile([1, BI * B_CAP], F32, tag="nrm_ps")
        _lag = int(os.environ.get("KLAG", "3"))

        _gq = int(os.environ.get("KGQ", "256"))  # L1 q-cols on gpsimd
        _pool_on = os.environ.get("KPOOL", "1") == "1"

        _ksch = int(os.environ.get("KSCH", "5"))  # ramp pairs w/ DVE exp

        _ksch2 = int(os.environ.get("KSCH2", "0"))  # drain pairs w/ DVE exp

        def stage_a(i, dt):
            # eq: [e|q] slabs, col = t*128 + s*64 + b
            eq = work.tile([128, 2 * R], BF16, tag="eq")
            eqv = eq[:].rearrange("p (t s b) -> p t s b", t=T_IMG, s=2)
            ct_tb = capT3[:, dt, :].rearrange("p (t b) -> p t b", t=T_IMG)
            _tail_s = (i == BI - 1 and dt >= NDT - _ksch2)
            if (i == 0 and dt < _ksch) or _tail_s:
                # ramp relief: Schraudolph exp on DVE (ACT is saturated by
                # stats/MLP here); bits = 184.664*(es*c+eb) + 16249 written
                # as int16 straight into the bf16 e-columns
                ebits = eqv[:, :, 0, :].bitcast(mybir.dt.int16)
                with nc.allow_low_precision(reason="ramp Schraudolph exp"):
                    nc.vector.tensor_scalar(ebits, ct_tb,
                                            esS3[:, dt, i:i + 1],
                                            ebS3[:, dt, i:i + 1],
                                            op0=ALU.mult, op1=ALU.add)
            else:
                nc.scalar.activation(eqv[:, :, 0, :], ct_tb, AF.Exp,
                                     bias=eb3[:, dt, i:i + 1],
                                     scale=es3[:, dt, i:i + 1])
            # L1 e-half on gpsimd: depends only on ACT exp, starts early
            a1 = work.tile([128, 18 * 128], BF16, tag="a1",
                           bufs=int(os.environ.get("KA1B", "4")))
            a1v = a1[:].rearrange("p (t s b) -> p t s b", t=18, s=2)
            eng0 = nc.gpsimd if _pool_on else nc.vector
            eng0.tensor_tensor(a1v[:, :, 0, :], eqv[:, 0:18, 0, :],
                               eqv[:, 18:36, 0, :], op=ALU.add)
            # q = e*cap on DVE (first KQP t-slabs on gpsimd)
            _qp = int(os.environ.get("KQP", "0")) // 64
            if _qp > 0:
                nc.gpsimd.tensor_tensor(eqv[:, 0:_qp, 1, :],
                                        eqv[:, 0:_qp, 0, :],
                                        ct_tb[:, 0:_qp, :], op=ALU.mult)
            nc.vector.tensor_tensor(eqv[:, _qp:T_IMG, 1, :],
                                    eqv[:, _qp:T_IMG, 0, :],
                                    ct_tb[:, _qp:T_IMG, :], op=ALU.mult)
            # L1 q-half split gpsimd/DVE
            eq1v = eqv[:, 0:18, 1, :]
            eq2v = eqv[:, 18:36, 1, :]
            a1q = a1v[:, :, 1, :]
            if _gq > 0 and _pool_on:
                gt = _gq // 64
                nc.gpsimd.tensor_tensor(a1q[:, 0:gt, :], eq1v[:, 0:gt, :],
                                        eq2v[:, 0:gt, :], op=ALU.add)
                nc.vector.tensor_tensor(a1q[:, gt:18, :], eq1v[:, gt:18, :],
                                        eq2v[:, gt:18, :], op=ALU.add)
            else:
                nc.vector.tensor_tensor(a1q, eq1v, eq2v, op=ALU.add)
            return i, dt, a1

        _gs2 = int(os.environ.get("KGS2", "0"))  # L2 cols on gpsimd

        def stage_b(state, a2all):
            i, dt, a1 = state
            # tree level 2: 18 -> 9 slabs, into the per-image collector;
            # the gpsimd share's consumer (per-image tail) runs pairs
            # later, so this handoff has slack (unlike the old per-pair a3)
            base = dt * 1152
            if _gs2 > 0:
                nc.gpsimd.tensor_tensor(a2all[:, base:base + _gs2],
                                        a1[:, 0:_gs2],
                                        a1[:, 1152:1152 + _gs2], op=ALU.add)
            nc.vector.tensor_tensor(a2all[:, base + _gs2:base + 1152],
                                    a1[:, _gs2:1152],
                                    a1[:, 1152 + _gs2:2304], op=ALU.add)

        _wact = os.environ.get("KWACT", "1") == "1"

        def make_c_steps(i, a2all):
            # batched tail across all dt: 9 -> (4+1) -> 2 -> 1 (+ slab 8),
            # returned as closures so steps interleave with later pairs
            a2v = a2all[:].rearrange("p (c s k) -> p c s k", c=NDT, s=9)
            a3 = work.tile([128, NDT * 512], BF16, tag="a3", bufs=1,
                           name=f"a3_{i}")
            a3v = a3[:].rearrange("p (c s k) -> p c s k", c=NDT, s=4)
            a4 = work.tile([128, NDT * 256], BF16, tag="a4", bufs=1,
                           name=f"a4_{i}")
            a4v = a4[:].rearrange("p (c s k) -> p c s k", c=NDT, s=2)
            a5 = work.tile([128, NDT * 128], BF16, tag="a5", bufs=1,
                           name=f"a5_{i}")
            a5v = a5[:].rearrange("p (c k) -> p c k", c=NDT)
            # bf16 tail: S1/S2 are already bf16-rounded through the tree,
            # so a bf16 final add/recip/sc costs one extra rounding and
            # halves the DVE cost (KF32TAIL=1 restores f32)
            TDT = F32 if os.environ.get("KF32TAIL", "0") == "1" else BF16
            s12all = work.tile([128, NDT * 128], TDT, tag="s12all", bufs=2,
                               name=f"s12all_{i}")
            s12sv = s12all[:].rearrange("p (c k) -> p c k", c=NDT)
            s12v = s12all[:].rearrange("p (c s b) -> p c s b", c=NDT, s=2)
            r1 = work.tile([128, NDT * B_CAP], TDT, tag="r1", bufs=1,
                           name=f"r1_{i}")
            r1v = r1[:].rearrange("p (c b) -> p c b", c=NDT)
            sc = work.tile([128, NDT * B_CAP], TDT, tag="sc", bufs=2,
                           name=f"sc_{i}")
            scv = sc[:].rearrange("p (c b) -> p c b", c=NDT)

            def c1():
                nc.vector.tensor_tensor(a3v, a2v[:, :, 0:4, :],
                                        a2v[:, :, 4:8, :], op=ALU.add)

            def c2():
                nc.vector.tensor_tensor(a4v, a3v[:, :, 0:2, :],
                                        a3v[:, :, 2:4, :], op=ALU.add)

            def c3():
                nc.vector.tensor_tensor(a5v, a4v[:, :, 0, :],
                                        a4v[:, :, 1, :], op=ALU.add)
                with nc.allow_low_precision(reason="S1/S2 already bf16"):
                    nc.vector.tensor_tensor(s12sv, a5v, a2v[:, :, 8, :],
                                            op=ALU.add)

            _scpool = os.environ.get("KSCPOOL", "0") == "1"

            def c4():
                with nc.allow_low_precision(reason="S1/S2 already bf16"):
                    nc.vector.reciprocal(r1v, s12v[:, :, 0, :])
                eng4 = nc.gpsimd if _scpool else nc.vector
                eng4.tensor_tensor(scv, s12v[:, :, 1, :], r1v,
                                   op=ALU.mult)

            def cw(dts):
                def run():
                    for dt in dts:
                        w_t = work.tile([128, B_CAP], BF16, tag="w", bufs=2)
                        w2_t = work.tile([128, B_CAP], BF16, tag="w2", bufs=2)
                        if _wact:
                            nc.scalar.activation(
                                w_t[:], scv[:, dt, :], AF.Identity,
                                bias=wb3[:, dt, i:i + 1],
                                scale=ws3[:, dt, i:i + 1])
                            nc.scalar.square(w2_t[:], w_t[:])
                        else:
                            nc.vector.tensor_scalar(
                                w_t[:], scv[:, dt, :], ws3[:, dt, i:i + 1],
                                wb3[:, dt, i:i + 1], op0=ALU.mult,
                                op1=ALU.add)
                            nc.vector.tensor_tensor(w2_t[:], w_t[:], w_t[:],
                                                    op=ALU.mult)
                        nc.tensor.matmul(
                            dot_ps[:, i * B_CAP:(i + 1) * B_CAP],
                            lhsT=imgrTb3[:, dt, i:i + 1], rhs=w_t[:],
                            start=(dt == 0), stop=(dt == NDT - 1))
                        nc.tensor.matmul(
                            nrm_ps[:, i * B_CAP:(i + 1) * B_CAP],
                            lhsT=ones_b[:], rhs=w2_t[:],
                            start=(dt == 0), stop=(dt == NDT - 1))
                return run

            _ncw = int(os.environ.get("KNCW", "2"))  # cw chunks per image
            step = NDT // _ncw
            cws = [cw(range(j, j + step)) for j in range(0, NDT, step)]
            return [c1, c2, c3, c4] + cws

        for _rep in range(reps_main):
            pending = []
            s12_by_i = {}
            done_by_i = {}
            c_steps = []

            def flush_one():
                state = pending.pop(0)
                fi = state[0]
                stage_b(state, s12_by_i[fi])
                done_by_i[fi] = done_by_i.get(fi, 0) + 1
                if done_by_i[fi] == NDT:
                    c_steps.extend(make_c_steps(fi, s12_by_i.pop(fi)))
                if c_steps:
                    c_steps.pop(0)()

            _dsp = int(os.environ.get("KDSP", "1"))  # deferred-stats spacing
            for i in range(BI):
                s12_by_i[i] = work.tile([128, NDT * 1152], BF16, tag="a2all",
                                        bufs=2, name=f"a2all_{i}")
                for dt in range(NDT):
                    pending.append(stage_a(i, dt))
                    if (_rep == 0 and deferred_stats
                            and dt % _dsp == 0):
                        deferred_stats.pop(0)()
                    if len(pending) > _lag:
                        flush_one()
            while pending:
                flush_one()
            for step in c_steps:
                step()

            # ---- epilogue: sims = dot * rsqrt(nrm) * (1/|v|) ----
            rr = small.tile([1, BI * B_CAP], F32, tag="rr")
            nc.vector.reciprocal(rr[:], nrm_ps[:])
            rsn = small.tile([1, BI * B_CAP], F32, tag="rsn")
            nc.scalar.sqrt(rsn[:], rr[:])
            prod = small.tile([1, BI * B_CAP], F32, tag="prod")
            nc.vector.tensor_tensor(prod[:], dot_ps[:], rsn[:], op=ALU.mult)
            res = small.tile([1, BI * B_CAP], F32, tag="res")
            rsr_b = rsr_row[:].rearrange("p (i u) -> p i u", u=1).broadcast_to([1, BI, B_CAP])
            nc.vector.tensor_tensor(
                res[:].rearrange("p (i b) -> p i b", i=BI),
                prod[:].rearrange("p (i b) -> p i b", i=BI),
                rsr_b, op=ALU.mult)
            nc.sync.dma_start(out=out_d[:].rearrange("i b -> (i b)"), in_=res[:])




def _get_compiled():
    global _COMPILED
    if _COMPILED is None:
        _COMPILED = _build()
    return _COMPILED


def _declare_io(nc):
    return (
        nc.dram_tensor("capT", [128, NDT * R], BF16, kind="ExternalInput"),
        nc.dram_tensor("img", [128, NDT * BI * T_IMG], F32,
                       kind="ExternalInput"),
        nc.dram_tensor("wg1", [D, H], F32, kind="ExternalInput"),
        nc.dram_tensor("wg2", [H, D], F32, kind="ExternalInput"),
        nc.dram_tensor("wb1", [D, H], F32, kind="ExternalInput"),
        nc.dram_tensor("wb2", [H, D], F32, kind="ExternalInput"),
        nc.dram_tensor("bg1", [H, 1], F32, kind="ExternalInput"),
        nc.dram_tensor("bb1", [H, 1], F32, kind="ExternalInput"),
        nc.dram_tensor("bg2p1", [128, NDT], F32, kind="ExternalInput"),
        nc.dram_tensor("bb2t", [128, NDT], F32, kind="ExternalInput"),
        nc.dram_tensor("o36", [T_IMG, 1], F32, kind="ExternalInput"),
        nc.dram_tensor("out", [BI, B_CAP], F32, kind="ExternalOutput"),
    )


def _build_repeated(reps):
    """Timing variant: run the compute `reps` times in one NEFF. With
    KREPMODE=main, phase A runs once and only the main loop repeats."""
    import os
    nc = bacc.Bacc("TRN2", target_bir_lowering=False, debug=False,
                   num_devices=N_CORES)
    tensors = _declare_io(nc)
    with TileContext(nc) as tc:
        if os.environ.get("KREPMODE") == "main":
            _emit(nc, tc, *tensors, reps_main=reps)
        else:
            for _ in range(reps):
                _emit(nc, tc, *tensors)
    nc.compile()
    return nc


def _in_maps(img_embed, cap_embed, Wg1, bg1, Wg2, bg2, Wb1, bb1, Wb2, bb2):
    import ml_dtypes
    # capT[p, dt, t, b] = cap[b, t, dt*128+p], t-major cols, bf16
    cap = np.asarray(cap_embed[:, :T_IMG, :], np.float32)       # [b, t, d]
    capT = cap.reshape(B_CAP, T_IMG, NDT, 128).transpose(3, 2, 1, 0)
    capT = np.ascontiguousarray(capT.reshape(128, NDT * R)).astype(
        ml_dtypes.bfloat16)
    shared = {
        "capT": capT,
        "wg1": np.ascontiguousarray(Wg1, np.float32),
        "wg2": np.ascontiguousarray(Wg2, np.float32),
        "wb1": np.ascontiguousarray(Wb1, np.float32),
        "wb2": np.ascontiguousarray(Wb2, np.float32),
        "bg1": np.ascontiguousarray(bg1.reshape(H, 1), np.float32),
        "bb1": np.ascontiguousarray(bb1.reshape(H, 1), np.float32),
        "bg2p1": np.ascontiguousarray((bg2 + 1.0).reshape(NDT, 128).T,
                                      np.float32),
        "bb2t": np.ascontiguousarray(bb2.reshape(NDT, 128).T, np.float32),
        "o36": np.full((T_IMG, 1), 1.0 / T_IMG, np.float32),
    }
    maps = []
    for c in range(N_CORES):
        m = dict(shared)
        # imgT[p, dt, i, t] = img[i, t, dt*128+p]
        im = np.asarray(img_embed[c * BI:(c + 1) * BI], np.float32)
        imT = im.reshape(BI, T_IMG, NDT, 128).transpose(3, 2, 0, 1)
        m["img"] = np.ascontiguousarray(
            imT.reshape(128, NDT * BI * T_IMG))
        maps.append(m)
    return maps


def kernel(img_embed, cap_embed, lens, Wg1, bg1, Wg2, bg2, Wb1, bb1, Wb2, bb2):
    del lens  # unused by the reference computation
    nc = _get_compiled()
    maps = _in_maps(np.asarray(img_embed), np.asarray(cap_embed),
                    np.asarray(Wg1), np.asarray(bg1), np.asarray(Wg2),
                    np.asarray(bg2), np.asarray(Wb1), np.asarray(bb1),
                    np.asarray(Wb2), np.asarray(bb2))
    import time as _time
    last = None
    for attempt in range(5):  # device occasionally needs runs to recover
        try:
            res = run_bass_kernel_spmd(nc, maps, core_ids=list(range(N_CORES)))
            out = np.concatenate(
                [res.results[c]["out"] for c in range(N_CORES)],
                axis=0).astype(np.float32)
            # sims are l2-normalized dots in [-1, 1]; non-finite values mean
            # the device glitched (observed once) — retry, don't return junk
            if np.isfinite(out).all():
                return out
            last = RuntimeError("non-finite kernel output; retrying")
        except Exception as e:
            last = e
        _time.sleep(10)
    raise last



# revision 16
# speedup vs baseline: 1.2750x; 1.2750x over previous
"""Trainium2 Bass kernel for nn_AdaptiveEmbeddingI2T.

Computes, for image-batch shard i on each of 8 NeuronCores:
  sims[i, b] = <img_vec_i, txt_vec_ib> with
  txt_vec_ib = l2norm_d( mean_t( softmax_t(10*(gam_id*xn_bdt+bet_id)) * (gam*xn+bet) ) )

Device-side algebra (softmax shift-invariance folds BN into an affine):
  exponent = es*cap + eb with es = 10*gam*rs, eb = -es*mu
  txt_vec = wscale*(S2/S1) + wbias, S1 = sum_t e, S2 = sum_t e*cap
  sims = (sum_d v*w) * rsqrt(sum_d w^2) * rsqrt(sum_d v^2)

Engine split per (i,dt) iteration (t-major cols, e/q contiguous tiles):
  ACT:    e = Exp(es*cap+eb)                          [128,2304]
  DVE:    q = e*cap; S2 tree levels 2+ (contiguous halving adds)
  GPSIMD: S2 tree level 1 (1152-col add)
  PE:     S1 = sum_t e via 18 accumulating transposes -> PSUM[128,128],
          DVE folds even/odd-t halves, PE transposes back to [d,b]
1/S1 via ACT Ln+Exp(-x) (table rsqrt/recip are blocked; DVE RECIPROCAL is
6.4ns/col). Final dot/norm via PE matmuls as before.

Sharding: image batch axis across 8 cores (8 images/core); cap + params
replicated; host concatenates the (8, 64) row blocks. Weights/img are
cast to bf16 host-side.
"""

import sys

if "/opt/trn_rl_repo" not in sys.path:
    sys.path.insert(0, "/opt/trn_rl_repo")

import numpy as np

import concourse.bacc as bacc
import concourse.mybir as mybir
from concourse.bass_utils import run_bass_kernel_spmd
from concourse.tile import TileContext

B_IMG, B_CAP, T_CAP, D = 64, 64, 64, 1024
H = 128
T_IMG = 36
EPS = 1e-5
N_CORES = 8
BI = B_IMG // N_CORES          # images per core
R = B_CAP * T_IMG              # 2304 caption cols per dt (t-major: col = t*64+b)
NDT = D // 128                 # 8 channel tiles

F32 = mybir.dt.float32
BF16 = mybir.dt.bfloat16

_COMPILED = None


def _build():
    nc = bacc.Bacc("TRN2", target_bir_lowering=False, debug=False,
                   num_devices=N_CORES)
    tensors = _declare_io(nc)
    with TileContext(nc) as tc:
        _emit(nc, tc, *tensors)
    nc.compile()
    return nc


def _emit(nc, tc, capT_d, img_d, wg1_d, wg2_d, wb1_d, wb2_d,
          bg1_d, bb1_d, bg2p1_d, bb2_d, ident_d, out_d, reps_main=1):
    AF = mybir.ActivationFunctionType
    ALU = mybir.AluOpType
    import os
    import contextlib
    ctx = contextlib.ExitStack()
    with ctx:
        const = ctx.enter_context(tc.tile_pool(name="const", bufs=1))
        stream = ctx.enter_context(tc.tile_pool(name="stream", bufs=2))
        imgs = ctx.enter_context(tc.tile_pool(name="imgs", bufs=2))
        work = ctx.enter_context(tc.tile_pool(name="work", bufs=2))
        small = ctx.enter_context(tc.tile_pool(name="small", bufs=1))
        actx = ctx.enter_context(contextlib.ExitStack())
        ppool = actx.enter_context(tc.tile_pool(name="psum", bufs=2, space="PSUM"))
        pacc = actx.enter_context(tc.tile_pool(name="pacc", bufs=1, space="PSUM"))

        # ---- small params ----
        bg1_s = const.tile([H, 1], F32, tag="bg1")
        nc.sync.dma_start(out=bg1_s[:], in_=bg1_d[:])
        bb1_s = const.tile([H, 1], F32, tag="bb1")
        nc.sync.dma_start(out=bb1_s[:], in_=bb1_d[:])
        bg2p1_s = const.tile([128, NDT], F32, tag="bg2p1")
        nc.sync.dma_start(out=bg2p1_s[:], in_=bg2p1_d[:])
        bb2_s = const.tile([128, NDT], F32, tag="bb2t")
        nc.sync.dma_start(out=bb2_s[:], in_=bb2_d[:])
        ident = const.tile([128, 128], BF16, tag="ident")
        nc.sync.dma_start(out=ident[:], in_=ident_d[:])

        # ---- MLP weights: bf16 direct from host ----
        wg1_b = const.tile([128, NDT * H], BF16, tag="wg1b")
        nc.sync.dma_start(out=wg1_b[:], in_=wg1_d[:])
        wb1_b = const.tile([128, NDT * H], BF16, tag="wb1b")
        nc.sync.dma_start(out=wb1_b[:], in_=wb1_d[:])
        wg2_b = const.tile([128, D], BF16, tag="wg2b")
        nc.sync.dma_start(out=wg2_b[:], in_=wg2_d[:])
        wb2_b = const.tile([128, D], BF16, tag="wb2b")
        nc.sync.dma_start(out=wb2_b[:], in_=wb2_d[:])

        ones_b = const.tile([128, 1], BF16, tag="onesb")
        nc.gpsimd.memset(ones_b[:], 1.0)

        # ---- image means from host-transposed bf16 img: one DVE reduce ----
        imgTb = imgs.tile([128, NDT * BI * T_IMG], BF16, tag="imgTb", bufs=1)
        nc.sync.dma_start(out=imgTb[:], in_=img_d[:])

        imgrT = const.tile([128, NDT * BI], F32, tag="imgrT")
        imgrTb = const.tile([128, NDT * BI], BF16, tag="imgrTb")
        imgrT3 = imgrT[:].rearrange("p (c i) -> p c i", c=NDT)
        imgrTb3 = imgrTb[:].rearrange("p (c i) -> p c i", c=NDT)
        nc.vector.tensor_reduce(
            imgrT[:],
            imgTb[:].rearrange("p (g t) -> p g t", t=T_IMG),
            axis=mybir.AxisListType.X, op=ALU.add)
        nc.vector.tensor_scalar_mul(imgrT[:], imgrT[:], 1.0 / T_IMG)
        nc.scalar.copy(imgrTb[:], imgrT[:])

        # ---- cap: DMA per dt chunk, stats pipelined behind chunks ----
        capTb = const.tile([128, NDT * R], BF16, tag="capTb")  # 4.6 MB
        capT3 = capTb[:].rearrange("p (c r) -> p c r", c=NDT)
        capTd3 = capT_d[:].rearrange("p (c r) -> p c r", c=NDT)
        for dt in range(NDT):
            nc.sync.dma_start(out=capT3[:, dt, :], in_=capTd3[:, dt, :])

        AXX = mybir.AxisListType.X
        musum = small.tile([128, NDT], F32, tag="musum")
        sqsum = small.tile([128, NDT], F32, tag="sqsum")
        mu = small.tile([128, NDT], F32, tag="mu")
        rs = small.tile([128, NDT], F32, tag="rs")
        tv = small.tile([128, NDT], F32, tag="tv")

        def stats_for(dt):
            # Sum(c): DVE halving-tree; Sum(c^2) via ACT Square+accum
            st1 = stream.tile([128, 1152], BF16, tag="st1")
            nc.vector.tensor_tensor(st1[:], capT3[:, dt, 0:1152],
                                    capT3[:, dt, 1152:2304], op=ALU.add)
            st2 = stream.tile([128, 576], BF16, tag="st2")
            nc.vector.tensor_tensor(st2[:], st1[:, 0:576], st1[:, 576:1152],
                                    op=ALU.add)
            st3 = stream.tile([128, 288], BF16, tag="st3")
            nc.vector.tensor_tensor(st3[:], st2[:, 0:288], st2[:, 288:576],
                                    op=ALU.add)
            st4 = stream.tile([128, 144], BF16, tag="st4")
            nc.vector.tensor_tensor(st4[:], st3[:, 0:144], st3[:, 144:288],
                                    op=ALU.add)
            st5 = stream.tile([128, 72], BF16, tag="st5")
            nc.vector.tensor_tensor(st5[:], st4[:, 0:72], st4[:, 72:144],
                                    op=ALU.add)
            st6 = stream.tile([128, 36], BF16, tag="st6")
            nc.vector.tensor_tensor(st6[:], st5[:, 0:36], st5[:, 36:72],
                                    op=ALU.add)
            nc.vector.tensor_reduce(
                musum[:, dt:dt + 1].rearrange("p (u o) -> p u o", u=1),
                st6[:].rearrange("p (u t) -> p u t", u=1), axis=AXX,
                op=ALU.add)
            scr2 = stream.tile([128, R], BF16, tag="scr2", bufs=1)
            nc.scalar.activation(scr2[:], capT3[:, dt, :], AF.Square,
                                 accum_out=sqsum[:, dt:dt + 1])

        def stats_fin(d0, d1):
            # var = E[x^2] - mu^2 ; rs = 1/sqrt(var+eps), dt range [d0, d1)
            nc.vector.tensor_scalar_mul(mu[:, d0:d1], musum[:, d0:d1],
                                        1.0 / R)
            nc.vector.tensor_tensor(tv[:, d0:d1], mu[:, d0:d1], mu[:, d0:d1],
                                    op=ALU.mult)
            nc.vector.tensor_scalar(sqsum[:, d0:d1], sqsum[:, d0:d1],
                                    1.0 / R, None, op0=ALU.mult)
            nc.vector.tensor_tensor(tv[:, d0:d1], sqsum[:, d0:d1],
                                    tv[:, d0:d1], op=ALU.subtract)
            nc.vector.tensor_scalar_add(tv[:, d0:d1], tv[:, d0:d1], EPS)
            nc.scalar.sqrt(tv[:, d0:d1], tv[:, d0:d1])
            nc.vector.reciprocal(rs[:, d0:d1], tv[:, d0:d1])

        # stats for the first half of dts, then MLPs, then first-half prep
        _shead = int(os.environ.get("KSHEAD", "5"))
        for dt in range(_shead):
            stats_for(dt)

        # ---- CBN MLPs -> per-(d,i) scales/biases ----
        wg1_b3 = wg1_b[:].rearrange("p (c h) -> p c h", c=NDT)
        wb1_b3 = wb1_b[:].rearrange("p (c h) -> p c h", c=NDT)

        def mlp_head(w1_b3, b1_s, w2_b, b2_s, name):
            h_ps = ppool.tile([H, BI], F32, tag="tr")
            for dt in range(NDT):
                nc.tensor.matmul(h_ps[:], lhsT=w1_b3[:, dt, :],
                                 rhs=imgrTb3[:, dt, :],
                                 start=(dt == 0), stop=(dt == NDT - 1))
            hT = small.tile([H, BI], BF16, tag=f"hT_{name}")
            nc.scalar.activation(hT[:], h_ps[:], AF.Relu, bias=b1_s[:], scale=1.0)
            outT = const.tile([128, NDT * BI], F32, tag=f"outT_{name}")
            outT3 = outT[:].rearrange("p (c i) -> p c i", c=NDT)
            for dt in range(NDT):
                o_ps = ppool.tile([128, BI], F32, tag="tr")
                nc.tensor.matmul(o_ps[:], lhsT=w2_b[:, dt * 128:(dt + 1) * 128],
                                 rhs=hT[:], start=True, stop=True)
                nc.scalar.activation(outT3[:, dt, :], o_ps[:], AF.Identity,
                                     bias=b2_s[:, dt:dt + 1], scale=1.0)
            return outT3

        gamT3 = mlp_head(wg1_b3, bg1_s, wg2_b, bg2p1_s, "g")
        betT3 = mlp_head(wb1_b3, bb1_s, wb2_b, bb2_s, "b")

        # escale = 10*gam*rs ; ebias = -escale*mu
        # wscale = gam*rs/36 ; wbias = (bet - gam*rs*mu)/36
        escale = const.tile([128, NDT * BI], F32, tag="escale")
        ebias = const.tile([128, NDT * BI], F32, tag="ebias")
        wscale = const.tile([128, NDT * BI], F32, tag="wscale")
        wbias = const.tile([128, NDT * BI], F32, tag="wbias")
        es3 = escale[:].rearrange("p (c i) -> p c i", c=NDT)
        eb3 = ebias[:].rearrange("p (c i) -> p c i", c=NDT)
        ws3 = wscale[:].rearrange("p (c i) -> p c i", c=NDT)
        wb3 = wbias[:].rearrange("p (c i) -> p c i", c=NDT)
        grs = stream.tile([128, NDT * BI], F32, tag="grs", bufs=1)
        tmp = stream.tile([128, NDT * BI], F32, tag="tmpb", bufs=1)
        grs3 = grs[:].rearrange("p (c i) -> p c i", c=NDT)
        tmp3 = tmp[:].rearrange("p (c i) -> p c i", c=NDT)
        rs_b = rs[:].rearrange("p (c u) -> p c u", u=1).broadcast_to(
            [128, NDT, BI])
        mu_b = mu[:].rearrange("p (c u) -> p c u", u=1).broadcast_to(
            [128, NDT, BI])

        esS = const.tile([128, NDT * BI], F32, tag="esS")
        ebS = const.tile([128, NDT * BI], F32, tag="ebS")
        esS3 = esS[:].rearrange("p (c i) -> p c i", c=NDT)
        ebS3 = ebS[:].rearrange("p (c i) -> p c i", c=NDT)

        def prep_for(d0, d1):
            nc.vector.tensor_tensor(grs3[:, d0:d1, :], gamT3[:, d0:d1, :],
                                    rs_b[:, d0:d1, :], op=ALU.mult)
            nc.vector.tensor_scalar_mul(es3[:, d0:d1, :], grs3[:, d0:d1, :],
                                        10.0)
            nc.vector.tensor_tensor(eb3[:, d0:d1, :], es3[:, d0:d1, :],
                                    mu_b[:, d0:d1, :], op=ALU.mult)
            nc.vector.tensor_scalar_mul(eb3[:, d0:d1, :], eb3[:, d0:d1, :],
                                        -1.0)
            nc.vector.tensor_scalar_mul(ws3[:, d0:d1, :], grs3[:, d0:d1, :],
                                        1.0 / 36.0)
            nc.vector.tensor_tensor(tmp3[:, d0:d1, :], grs3[:, d0:d1, :],
                                    mu_b[:, d0:d1, :], op=ALU.mult)
            nc.vector.tensor_tensor(tmp3[:, d0:d1, :], betT3[:, d0:d1, :],
                                    tmp3[:, d0:d1, :], op=ALU.subtract)
            nc.vector.tensor_scalar_mul(wb3[:, d0:d1, :], tmp3[:, d0:d1, :],
                                        1.0 / 36.0)
            # Schraudolph coefficients: bits = 184.664*z + 16249
            nc.vector.tensor_scalar_mul(esS3[:, d0:d1, :], es3[:, d0:d1, :],
                                        184.664)
            nc.vector.tensor_scalar(ebS3[:, d0:d1, :], eb3[:, d0:d1, :],
                                    184.664, 16249.0, op0=ALU.mult,
                                    op1=ALU.add)

        stats_fin(0, _shead)
        prep_for(0, _shead)

        def _tail_stats(dt):
            def run():
                stats_for(dt)
                if dt == NDT - 1:
                    stats_fin(_shead, NDT)
                    prep_for(_shead, NDT)
            return run

        deferred_stats = [_tail_stats(dt) for dt in range(_shead, NDT)]

        # rnorm: 1/||v_i|| via accumulating [1,1] matmuls
        nrm2_ps = pacc.tile([1, BI], F32, tag="nrm2_ps")
        for i in range(BI):
            for dt in range(NDT):
                nc.tensor.matmul(
                    nrm2_ps[:, i:i + 1],
                    lhsT=imgrT3[:, dt, i:i + 1], rhs=imgrT3[:, dt, i:i + 1],
                    start=(dt == 0), stop=(dt == NDT - 1))
        nrm_row = small.tile([1, BI], F32, tag="nrm_row")
        nc.scalar.sqrt(nrm_row[:], nrm2_ps[:])
        rsr_row = small.tile([1, BI], F32, tag="rsr_row")
        nc.vector.reciprocal(rsr_row[:], nrm_row[:])

        # ---- main loop over (i, dt) ----
        actx.close()  # release phase-A psum banks
        pmain = ctx.enter_context(tc.tile_pool(name="pmain", bufs=1,
                                               space="PSUM"))
        ptr = ctx.enter_context(tc.tile_pool(name="ptr", bufs=2, space="PSUM"))
        dot_ps = pmain.tile([1, BI * B_CAP], F32, tag="dot_ps")
        nrm_ps = pmain.tile([1, BI * B_CAP], F32, tag="nrm_ps")

        _ksch = int(os.environ.get("KSCH", "5"))     # ramp iters (i==0) w/ DVE exp
        _lagA = int(os.environ.get("KLAGA", "2"))
        _lagB = int(os.environ.get("KLAGB", "2"))
        _l1gps = int(os.environ.get("KL1GPS", "2"))
        _epath = os.environ.get("KEPATH", "pe")      # 'pe' | 'dve'
        _qpath = os.environ.get("KQPATH", "dve")      # 'pe' | 'dve'
        _ebufs = int(os.environ.get("KEBUFS", "3"))

        def stage_exp(i, dt):
            e = work.tile([128, R], BF16, tag="e", bufs=_ebufs)
            ct = capT3[:, dt, :]
            if i == 0 and dt < _ksch:
                ebits = e[:].bitcast(mybir.dt.int16)
                with nc.allow_low_precision(reason="ramp Schraudolph exp"):
                    nc.vector.tensor_scalar(ebits, ct,
                                            esS3[:, dt, i:i + 1],
                                            ebS3[:, dt, i:i + 1],
                                            op0=ALU.mult, op1=ALU.add)
            else:
                nc.scalar.activation(e[:], ct, AF.Exp,
                                     bias=eb3[:, dt, i:i + 1],
                                     scale=es3[:, dt, i:i + 1])
            return e

        _kcw = int(os.environ.get("KCW", "128"))     # PE reduce chunk width

        def pe_reduce(src, tag):
            # sum over t via accumulating identity-matmuls (PSUM f32):
            # P[d, m] = sum_c src[d, c*KCW + m]; identity is stationary.
            # m spans KCW/64 t-positions x 64 b; later fold adds them.
            n = R // _kcw
            P = ptr.tile([128, _kcw], F32, tag=tag)
            for c in range(n):
                nc.tensor.matmul(P[:], lhsT=ident[:],
                                 rhs=src[:, c * _kcw:(c + 1) * _kcw],
                                 start=(c == 0), stop=(c == n - 1))
            return P

        def dve_tree(src, out_slice, pfx, dt=0):
            # contiguous t-halving: 36->18 (in src) ->9 ->(4+1)->2->1
            a1 = work.tile([128, 1152], BF16, tag=f"{pfx}1", bufs=2)
            use_gps = _l1gps == 1 or (_l1gps == 2 and dt % 2 == 0)
            eng = nc.gpsimd if use_gps else nc.vector
            eng.tensor_tensor(a1[:], src[:, 0:1152], src[:, 1152:2304],
                              op=ALU.add)
            a2 = work.tile([128, 576], BF16, tag=f"{pfx}2", bufs=2)
            nc.vector.tensor_tensor(a2[:], a1[:, 0:576], a1[:, 576:1152],
                                    op=ALU.add)
            a3 = work.tile([128, 256], BF16, tag=f"{pfx}3", bufs=2)
            nc.vector.tensor_tensor(a3[:], a2[:, 0:256], a2[:, 256:512],
                                    op=ALU.add)
            a4 = work.tile([128, 128], BF16, tag=f"{pfx}4", bufs=2)
            nc.vector.tensor_tensor(a4[:], a3[:, 0:128], a3[:, 128:256],
                                    op=ALU.add)
            sh = work.tile([128, 64], BF16, tag=f"{pfx}5", bufs=2)
            nc.vector.tensor_tensor(sh[:], a4[:, 0:64], a4[:, 64:128],
                                    op=ALU.add)
            with nc.allow_low_precision(reason="bf16 tree"):
                nc.vector.tensor_tensor(out_slice, sh[:], a2[:, 512:576],
                                        op=ALU.add)

        def pe_fold(P, out_slice, tag):
            # PSUM -> SBUF copy (bf16), then fold the KCW/64 t-positions
            # by contiguous halving TTs (same partitions, legal)
            w = _kcw
            with nc.allow_low_precision(reason="S bf16"):
                if w == 64:
                    nc.vector.tensor_copy(out_slice, P[:])
                    return
                pc = work.tile([128, w], BF16, tag=f"f{tag}", bufs=2)
                nc.vector.tensor_copy(pc[:], P[:])
                while w > 128:
                    h = w // 2
                    nc.vector.tensor_tensor(pc[:, 0:h], pc[:, 0:h],
                                            pc[:, h:w], op=ALU.add)
                    w = h
                nc.vector.tensor_tensor(out_slice, pc[:, 0:64],
                                        pc[:, 64:128], op=ALU.add)

        def stage_front(i, dt, e, s_tiles):
            # q = e*cap (DVE); transposes (PE) for S1/S2
            ct = capT3[:, dt, :]
            q = work.tile([128, R], BF16, tag="q", bufs=2)
            Pe = pe_reduce(e, "Pe") if _epath == "pe" else None
            nc.vector.tensor_tensor(q[:], e[:], ct, op=ALU.mult)
            Pq = pe_reduce(q, "Pq") if _qpath == "pe" else None
            return e, q, Pe, Pq

        def stage_back(i, dt, front, s_tiles):
            S1all, S2all = s_tiles
            e, q, Pe, Pq = front
            if _qpath == "pe":
                pe_fold(Pq, S2all[:, dt * 64:(dt + 1) * 64], "q")
            else:
                dve_tree(q, S2all[:, dt * 64:(dt + 1) * 64], "a", dt)
            if _epath == "pe":
                pe_fold(Pe, S1all[:, dt * 64:(dt + 1) * 64], "e")
            else:
                dve_tree(e, S1all[:, dt * 64:(dt + 1) * 64], "b", dt)

        _krecip = os.environ.get("KRECIP", "newton")

        def make_tail(i, s_tiles):
            S1all, S2all = s_tiles
            r0 = work.tile([128, NDT * 64], BF16, tag="r0", bufs=2,
                           name=f"r0_{i % 2}")
            nt = work.tile([128, NDT * 64], BF16, tag="nt", bufs=2,
                           name=f"nt_{i % 2}")
            rinv = work.tile([128, NDT * 64], BF16, tag="rinv", bufs=2,
                             name=f"rinv_{i % 2}")
            sall = work.tile([128, NDT * 64], BF16, tag="sall", bufs=2,
                             name=f"sall_{i % 2}")
            wall = work.tile([128, NDT * 64], BF16, tag="wall", bufs=2,
                             name=f"wall_{i % 2}")
            w2all = work.tile([128, NDT * 64], BF16, tag="w2all", bufs=2,
                              name=f"w2all_{i % 2}")

            def t1():
                if _krecip == "newton":
                    # seed r0 ~= 1/S1 in bf16 bit domain: bits(1/x) ~=
                    # 32505 - bits(x) (valid: S1 > 0)
                    with nc.allow_low_precision(reason="recip seed"):
                        nc.vector.tensor_scalar(
                            r0[:].bitcast(mybir.dt.int16),
                            S1all[:].bitcast(mybir.dt.int16),
                            -1.0, 32505.0, op0=ALU.mult, op1=ALU.add)
                else:
                    nc.vector.reciprocal(rinv[:], S1all[:])

            def t2():
                if _krecip == "newton":
                    # one Newton step: r1 = r0*(2 - S1*r0)
                    with nc.allow_low_precision(reason="newton"):
                        nc.vector.tensor_tensor(nt[:], S1all[:], r0[:],
                                                op=ALU.mult)
                        nc.vector.tensor_scalar(nt[:], nt[:], -1.0, 2.0,
                                                op0=ALU.mult, op1=ALU.add)
                        nc.vector.tensor_tensor(rinv[:], r0[:], nt[:],
                                                op=ALU.mult)

            def t3():
                with nc.allow_low_precision(reason="sigma bf16"):
                    nc.vector.tensor_tensor(sall[:], S2all[:], rinv[:],
                                            op=ALU.mult)

            def t4():
                for dt in range(NDT):
                    nc.vector.tensor_scalar(
                        wall[:, dt * 64:(dt + 1) * 64],
                        sall[:, dt * 64:(dt + 1) * 64],
                        ws3[:, dt, i:i + 1], wb3[:, dt, i:i + 1],
                        op0=ALU.mult, op1=ALU.add)

            def t5():
                nc.scalar.square(w2all[:], wall[:])

            def t6():
                for dt in range(NDT):
                    nc.tensor.matmul(
                        dot_ps[:, i * B_CAP:(i + 1) * B_CAP],
                        lhsT=imgrTb3[:, dt, i:i + 1],
                        rhs=wall[:, dt * 64:(dt + 1) * 64],
                        start=(dt == 0), stop=(dt == NDT - 1))
                    nc.tensor.matmul(
                        nrm_ps[:, i * B_CAP:(i + 1) * B_CAP],
                        lhsT=ones_b[:],
                        rhs=w2all[:, dt * 64:(dt + 1) * 64],
                        start=(dt == 0), stop=(dt == NDT - 1))

            return [t1, t2, t3, t4, t5, t6]

        for _rep in range(reps_main):
            pendA = []   # (i, dt, e) awaiting front
            pendB = []   # (i, dt, front, s_tiles) awaiting back
            tails = []
            s_by_i = {}

            def flushA():
                i, dt, e = pendA.pop(0)
                st = s_by_i[i]
                front = stage_front(i, dt, e, st)
                pendB.append((i, dt, front, st))

            def flushB():
                i, dt, front, st = pendB.pop(0)
                stage_back(i, dt, front, st)
                if dt == NDT - 1:
                    tails.extend(make_tail(i, st))

            def tick():
                if len(pendA) > _lagA:
                    flushA()
                if len(pendB) > _lagB:
                    flushB()
                if tails:
                    tails.pop(0)()

            _dsp = int(os.environ.get("KDSP", "1"))
            for i in range(BI):
                s_by_i[i] = (
                    work.tile([128, NDT * 64], BF16, tag="S1all", bufs=2,
                              name=f"S1all_{i % 2}"),
                    work.tile([128, NDT * 64], BF16, tag="S2all", bufs=2,
                              name=f"S2all_{i % 2}"),
                )
                for dt in range(NDT):
                    pendA.append((i, dt, stage_exp(i, dt)))
                    if (_rep == 0 and deferred_stats and dt % _dsp == 0):
                        deferred_stats.pop(0)()
                    tick()
            while pendA or pendB or tails:
                if pendA:
                    flushA()
                if pendB:
                    flushB()
                if tails:
                    tails.pop(0)()

            # ---- epilogue: sims = dot * rsqrt(nrm) * (1/|v|) ----
            rln2 = small.tile([1, BI * B_CAP], F32, tag="rln2")
            nc.scalar.activation(rln2[:], nrm_ps[:], AF.Ln)
            rsn = small.tile([1, BI * B_CAP], F32, tag="rsn")
            nc.scalar.activation(rsn[:], rln2[:], AF.Exp, scale=-0.5)
            prod = small.tile([1, BI * B_CAP], F32, tag="prod")
            nc.vector.tensor_tensor(prod[:], dot_ps[:], rsn[:], op=ALU.mult)
            res = small.tile([1, BI * B_CAP], F32, tag="res")
            rsr_b = rsr_row[:].rearrange("p (i u) -> p i u", u=1).broadcast_to(
                [1, BI, B_CAP])
            nc.vector.tensor_tensor(
                res[:].rearrange("p (i b) -> p i b", i=BI),
                prod[:].rearrange("p (i b) -> p i b", i=BI),
                rsr_b, op=ALU.mult)
            nc.sync.dma_start(out=out_d[:].rearrange("i b -> (i b)"),
                              in_=res[:])


def _get_compiled():
    global _COMPILED
    if _COMPILED is None:
        _COMPILED = _build()
    return _COMPILED


def _declare_io(nc):
    return (
        nc.dram_tensor("capT", [128, NDT * R], BF16, kind="ExternalInput"),
        nc.dram_tensor("img", [128, NDT * BI * T_IMG], BF16,
                       kind="ExternalInput"),
        nc.dram_tensor("wg1", [128, NDT * H], BF16, kind="ExternalInput"),
        nc.dram_tensor("wg2", [128, D], BF16, kind="ExternalInput"),
        nc.dram_tensor("wb1", [128, NDT * H], BF16, kind="ExternalInput"),
        nc.dram_tensor("wb2", [128, D], BF16, kind="ExternalInput"),
        nc.dram_tensor("bg1", [H, 1], F32, kind="ExternalInput"),
        nc.dram_tensor("bb1", [H, 1], F32, kind="ExternalInput"),
        nc.dram_tensor("bg2p1", [128, NDT], F32, kind="ExternalInput"),
        nc.dram_tensor("bb2t", [128, NDT], F32, kind="ExternalInput"),
        nc.dram_tensor("ident", [128, 128], BF16, kind="ExternalInput"),
        nc.dram_tensor("out", [BI, B_CAP], F32, kind="ExternalOutput"),
    )


def _build_repeated(reps):
    """Timing variant: run the compute `reps` times in one NEFF. With
    KREPMODE=main, phase A runs once and only the main loop repeats."""
    import os
    nc = bacc.Bacc("TRN2", target_bir_lowering=False, debug=False,
                   num_devices=N_CORES)
    tensors = _declare_io(nc)
    with TileContext(nc) as tc:
        if os.environ.get("KREPMODE") == "main":
            _emit(nc, tc, *tensors, reps_main=reps)
        else:
            for _ in range(reps):
                _emit(nc, tc, *tensors)
    nc.compile()
    return nc


def _in_maps(img_embed, cap_embed, Wg1, bg1, Wg2, bg2, Wb1, bb1, Wb2, bb2):
    import ml_dtypes
    # capT[p, dt, t, b] = cap[b, t, dt*128+p], t-major cols, bf16
    cap = np.asarray(cap_embed[:, :T_IMG, :], np.float32)       # [b, t, d]
    capT = cap.reshape(B_CAP, T_IMG, NDT, 128).transpose(3, 2, 1, 0)
    capT = np.ascontiguousarray(capT.reshape(128, NDT * R)).astype(
        ml_dtypes.bfloat16)

    def pack_w1(w):
        # [D, H] -> [p=128, (c, h)]
        w = np.asarray(w, np.float32).reshape(NDT, 128, H).transpose(1, 0, 2)
        return np.ascontiguousarray(w.reshape(128, NDT * H)).astype(
            ml_dtypes.bfloat16)

    shared = {
        "capT": capT,
        "wg1": pack_w1(Wg1),
        "wg2": np.ascontiguousarray(Wg2, np.float32).astype(
            ml_dtypes.bfloat16),
        "wb1": pack_w1(Wb1),
        "wb2": np.ascontiguousarray(Wb2, np.float32).astype(
            ml_dtypes.bfloat16),
        "bg1": np.ascontiguousarray(bg1.reshape(H, 1), np.float32),
        "bb1": np.ascontiguousarray(bb1.reshape(H, 1), np.float32),
        "bg2p1": np.ascontiguousarray((bg2 + 1.0).reshape(NDT, 128).T,
                                      np.float32),
        "bb2t": np.ascontiguousarray(bb2.reshape(NDT, 128).T, np.float32),
        "ident": np.eye(128, dtype=ml_dtypes.bfloat16),
    }
    maps = []
    for c in range(N_CORES):
        m = dict(shared)
        # imgT[p, dt, i, t] = img[i, t, dt*128+p]
        im = np.asarray(img_embed[c * BI:(c + 1) * BI], np.float32)
        imT = im.reshape(BI, T_IMG, NDT, 128).transpose(3, 2, 0, 1)
        m["img"] = np.ascontiguousarray(
            imT.reshape(128, NDT * BI * T_IMG)).astype(ml_dtypes.bfloat16)
        maps.append(m)
    return maps


def kernel(img_embed, cap_embed, lens, Wg1, bg1, Wg2, bg2, Wb1, bb1, Wb2, bb2):
    del lens  # unused by the reference computation
    nc = _get_compiled()
    maps = _in_maps(np.asarray(img_embed), np.asarray(cap_embed),
                    np.asarray(Wg1), np.asarray(bg1), np.asarray(Wg2),
                    np.asarray(bg2), np.asarray(Wb1), np.asarray(bb1),
                    np.asarray(Wb2), np.asarray(bb2))
    import time as _time
    last = None
    for attempt in range(5):  # device occasionally needs runs to recover
        try:
            res = run_bass_kernel_spmd(nc, maps, core_ids=list(range(N_CORES)))
            out = np.concatenate(
                [res.results[c]["out"] for c in range(N_CORES)],
                axis=0).astype(np.float32)
            # sims are l2-normalized dots in [-1, 1]; non-finite values mean
            # the device glitched (observed once) — retry, don't return junk
            if np.isfinite(out).all():
                return out
            last = RuntimeError("non-finite kernel output; retrying")
        except Exception as e:
            last = e
        _time.sleep(10)
    raise last
